# revision 6
# baseline (speedup 1.0000x reference)
"""7x7 'same' 2D convolution over [128, 512, 512] f32, data-parallel on 8 NeuronCores.

Banded-Toeplitz formulation on the TensorEngine with 64x64 array
packing: the PE array runs as 4 independent 64x64 tiles
(tile_position=(64r, 64g)), each computing a 58-row output block
    out[i0+m, j] = sum_v sum_{r'} T_v[r', m] * xpad[i0+r', j+v]
with T_v[r', m] = w[r'-m, v] (band, r'<64, m<58). The 7 column taps (v)
accumulate into PSUM; tile (s, r2, g2) covers out rows
232s + 116g2 + 58r2 + [0,58) and writes PSUM bank (s, r2), partitions
[64g2, 64g2+58). One 4-tile tap sweep streams in one N=512 matmul
time; 8 tiles cover rows 0..463 of an image. Rows 464..511 of four
consecutive images batch into one 4-tile "tail" group whose moving
data sits on partition strip j%2 only (no duplicated tail rows).

A burst of dummy matmuls on scratch data runs during the initial DMA
lead-in so the PE HAM clock-gate reaches 8/8 (2.4 GHz) before the
first real matmul; without it the first ~4us of real matmuls run at
1.2 GHz.

Inputs are cast to fp16 and pre-staged host-side into the SBUF slab
layout (partition 64r+p, slab q<4, col) = padded row 116q + 58r + p;
the 54 tail rows load separately into partitions [64*(img%2), +54).
Accumulation is fp32; PSUM banks are evacuated full-width (VectorE/
ScalarE alternating) into [128, 2048] bf16 staging tiles and stored
with plain partition-range DMAs that carry only the valid 58 (48) of
each 64-partition strip, so output HBM traffic is exactly the
512x512 bf16 image.
"""

import numpy as np

B, H, W = 128, 512, 512
KS = 7
PAD = (KS - 1) // 2          # 3
HP = H + 2 * PAD             # 518
N_CORES = 8
PER_CORE = B // N_CORES      # 16
TS = 58                      # output rows per 64x64 tile (64 - 6)
TAILM = H - 8 * TS           # 48 tail output rows per image
TAILK = TAILM + KS - 1       # 54
NWARM = 9                    # dummy matmuls for PE clock warm-up


def _build_program():
    import concourse.bass as bass
    import concourse.tile as tile
    from concourse import bacc, mybir

    f16 = mybir.dt.float16
    bf16 = mybir.dt.bfloat16
    f32 = mybir.dt.float32

    nc = bacc.Bacc("TRN2", target_bir_lowering=False, debug=False,
                   num_devices=N_CORES)
    x_ext = nc.declare_dram_parameter("x", [PER_CORE, 128, 4 * HP], f16,
                                      isOutput=False)
    xt_ext = nc.declare_dram_parameter("xt", [PER_CORE, TAILK, HP], f16,
                                       isOutput=False)
    t_ext = nc.declare_dram_parameter("toep", [128, KS * TS], f16,
                                      isOutput=False)
    # og[img, g, p, 2s+r, w]: out row 232s + 116g + 58r + p  (p < 58)
    og_ext = nc.declare_dram_parameter("og", [PER_CORE, 2, TS, 4, W],
                                       bf16, isOutput=True)
    # ot[tg, g, p, r, w]: img 4tg + 2g + r, out row 464 + p  (p < 48)
    ot_ext = nc.declare_dram_parameter("ot", [PER_CORE // 4, 2, TAILM, 2, W],
                                       bf16, isOutput=True)

    with tile.TileContext(nc) as tc:
        with (
            tc.tile_pool(name="toep", bufs=1) as toep_pool,
            tc.tile_pool(name="xa", bufs=4) as xa_pool,
            tc.tile_pool(name="xb", bufs=8) as xb_pool,
            tc.tile_pool(name="xt", bufs=8) as xt_pool,
            tc.tile_pool(name="psum", bufs=8, space="PSUM") as psum_pool,
            tc.tile_pool(name="outs", bufs=3) as out_pool,
            tc.tile_pool(name="outt", bufs=2) as outt_pool,
            tc.tile_pool(name="scr", bufs=1) as scr_pool,
        ):
            # PE warm-up: keep the array busy through the HAM activity
            # window while the first input DMAs are in flight.
            scr = scr_pool.tile([128, W], f16, name="scr")
            nc.vector.memset(scr[:], 0.0)
            wps = psum_pool.tile([128, W], f32, name="wps", tag="acc")
            for i in range(NWARM):
                nc.tensor.matmul(
                    wps[0:TS, :], scr[0:64, 0:TS], scr[0:64, :],
                    start=True, stop=True, tile_position=(0, 0),
                )

            toep_sb = toep_pool.tile([128, KS * TS], f16)
            nc.sync.dma_start(out=toep_sb[:], in_=t_ext[:])

            def evac(ps, osb, blk, idx):
                if idx % 2 == 0:
                    nc.vector.tensor_copy(osb[:, W * blk:W * (blk + 1)],
                                          ps[:])
                else:
                    nc.scalar.copy(osb[:, W * blk:W * (blk + 1)], ps[:])

            stages = {}
            tails = {}
            for img in range(PER_CORE):
                # (partition 64r+p, slab q) = padded row 116q + 58r + p.
                # Slabs 0-1 and 2-3 are separate tiles so the s=0 matmuls
                # depend only on the first (smaller) load.
                st_a = xa_pool.tile([128, 2 * HP], f16, name="sta",
                                    tag="sta")
                nc.sync.dma_start(out=st_a[:], in_=x_ext[img, :, :2 * HP])
                st_b = xb_pool.tile([128, 2 * HP], f16, name="stb",
                                    tag="stb")
                nc.sync.dma_start(out=st_b[:], in_=x_ext[img, :, 2 * HP:])
                # tail rows 464..518 on partition strip img%2 only
                st_t = xt_pool.tile([128, HP], f16, name="stt", tag="stt")
                hh = 64 * (img % 2)
                nc.sync.dma_start(out=st_t[hh:hh + TAILK, :],
                                  in_=xt_ext[img])
                stages[img] = st_b
                tails[img] = st_t

                osb = out_pool.tile([128, 4 * W], bf16, name="o",
                                    tag="osb")
                # s-groups sequential: only 2 PSUM banks live per group,
                # so allocation never stalls on evacuation of 4 banks.
                for s in range(2):
                    stage = st_a if s == 0 else st_b
                    ps = [psum_pool.tile([128, W], f32, name=f"ps{r}",
                                         tag="acc") for r in range(2)]
                    for v in range(KS):
                        for g in range(2):
                            for r in range(2):
                                nc.tensor.matmul(
                                    ps[r][64 * g:64 * g + TS, :],
                                    toep_sb[64 * r:64 * r + 64,
                                            TS * v:TS * (v + 1)],
                                    stage[64 * r:64 * r + 64,
                                          g * HP + v:g * HP + v + W],
                                    start=(v == 0),
                                    stop=(v == KS - 1),
                                    tile_position=(64 * r, 64 * g),
                                )
                    for r in range(2):
                        evac(ps[r], osb, 2 * s + r, img * 4 + 2 * s + r)

                    if s == 0 and img % 4 == 3:
                        # tail group between the two s-groups so its
                        # evacuations interleave with s=1 compute
                        tg = img // 4
                        pst = [psum_pool.tile([128, W], f32,
                                              name=f"pt{r}", tag="acc")
                               for r in range(2)]
                        for v in range(KS):
                            for j in range(4):
                                r, g = j % 2, j // 2
                                nc.tensor.matmul(
                                    pst[r][64 * g:64 * g + TAILM, :],
                                    toep_sb[64 * r:64 * r + TAILK,
                                            TS * v:TS * v + TAILM],
                                    tails[4 * tg + j][
                                        64 * r:64 * r + TAILK,
                                        v:v + W],
                                    start=(v == 0),
                                    stop=(v == KS - 1),
                                    tile_position=(64 * r, 64 * g),
                                )
                        otb = outt_pool.tile([128, 2 * W], bf16,
                                             name="p", tag="otb")
                        for r in range(2):
                            evac(pst[r], otb, r, img * 4 + r)
                        for g in range(2):
                            ring = nc.gpsimd if g == 0 else nc.scalar
                            ring.dma_start(
                                out=ot_ext[tg, g],
                                in_=otb[64 * g:64 * g + TAILM, :])
                        stages = {}
                        tails = {}

                for g in range(2):
                    ring = nc.gpsimd if (img + g) % 2 == 0 else nc.scalar
                    ring.dma_start(out=og_ext[img, g],
                                   in_=osb[64 * g:64 * g + TS, :])
    nc.finalize()
    return nc


def _host_prep(x, w):
    x = np.asarray(x, dtype=np.float32)
    w = np.asarray(w, dtype=np.float32)
    xpad = np.zeros((B, HP, HP), dtype=np.float16)
    xpad[:, PAD:PAD + H, PAD:PAD + W] = x
    # main slabs: (p, q<4) -> padded row 116q + 58*(p//64) + p%64
    p = np.arange(128)
    q = np.arange(4)
    ridx = 116 * q[None, :] + 58 * (p[:, None] // 64) + (p[:, None] % 64)
    xmain = np.ascontiguousarray(
        xpad[:, ridx, :].reshape(B, 128, 4 * HP))
    # tail rows 464..517 (54 = TAILK), compact
    xtail = np.ascontiguousarray(xpad[:, 464:464 + TAILK, :])
    # Toeplitz band [64, 58] per tap, replicated on both partition strips
    toep = np.zeros((128, KS * TS), dtype=np.float16)
    w16 = w.astype(np.float16)
    idx = np.arange(TS)
    for st in range(2):
        for v in range(KS):
            for d in range(KS):
                toep[64 * st + idx + d, TS * v + idx] = w16[d, v]
    return xmain, xtail, toep


def _execute(x, w, **run_kwargs):
    from concourse.bass_utils import run_bass_kernel_spmd

    xmain, xtail, toep = _host_prep(x, w)
    nc = _build_program()
    in_maps = [
        {"x": xmain[c * PER_CORE:(c + 1) * PER_CORE],
         "xt": xtail[c * PER_CORE:(c + 1) * PER_CORE],
         "toep": toep}
        for c in range(N_CORES)
    ]
    last_err = None
    for _attempt in range(3):
        try:
            res = run_bass_kernel_spmd(nc, in_maps,
                                       core_ids=list(range(N_CORES)),
                                       **run_kwargs)
            break
        except Exception as e:  # transient NRT execute flakes -> retry
            last_err = e
    else:
        raise last_err

    # og[img, g, p, 2s+r, w] -> out row 232s + 116g + 58r + p
    g_i, p_i, b_i = np.meshgrid(np.arange(2), np.arange(TS), np.arange(4),
                                indexing="ij")
    rm_og = (232 * (b_i // 2) + 116 * g_i + 58 * (b_i % 2) + p_i).ravel()
    out = np.empty((B, H, W), dtype=np.float32)
    for c in range(N_CORES):
        sl = slice(c * PER_CORE, (c + 1) * PER_CORE)
        og = np.asarray(res.results[c]["og"], dtype=np.float32)
        ot = np.asarray(res.results[c]["ot"], dtype=np.float32)
        out[sl][:, rm_og, :] = og.reshape(PER_CORE, 2 * TS * 4, W)
        # ot[tg, g, p, r, w] -> img 4tg + 2g + r, row 464 + p
        out[sl, 8 * TS:, :] = ot.transpose(0, 1, 3, 2, 4).reshape(
            PER_CORE, TAILM, W)
    return out, res


def kernel(x, w):
    out, _ = _execute(x, w)
    return out


# revision 9
# speedup vs baseline: 1.0758x; 1.0758x over previous
"""7x7 'same' 2D convolution over [128, 512, 512] f32, data-parallel on 8 NeuronCores.

Banded-Toeplitz formulation on the TensorEngine with 64x64 array
packing: the PE array runs as 4 independent 64x64 tiles
(tile_position=(64r, 64g)), each computing a 58-row output block
    out[i0+m, j] = sum_v sum_{r'} T_v[r', m] * xpad[i0+r', j+v]
with T_v[r', m] = w[r'-m, v] (band, r'<64, m<58). The 7 column taps (v)
accumulate into PSUM; tile (s, r2, g2) covers out rows
232s + 116g2 + 58r2 + [0,58) and writes PSUM bank (s, r2), partitions
[64g2, 64g2+58). One 4-tile tap sweep streams in one N=512 matmul
time; 8 tiles cover rows 0..463 of an image. Rows 464..511 of four
consecutive images batch into one 4-tile "tail" group whose moving
data sits on partition strip j%2 only (no duplicated tail rows).

A burst of dummy matmuls on scratch data runs during the initial DMA
lead-in so the PE HAM clock-gate reaches 8/8 (2.4 GHz) before the
first real matmul; without it the first ~4us of real matmuls run at
1.2 GHz.

Inputs are cast to fp16 and pre-staged host-side into the SBUF slab
layout (partition 64r+p, slab q<4, col) = padded row 116q + 58r + p;
the 54 tail rows load separately into partitions [64*(img%2), +54).
Accumulation is fp32; PSUM banks are evacuated full-width (VectorE/
ScalarE alternating) into [128, 2048] bf16 staging tiles and stored
with plain partition-range DMAs that carry only the valid 58 (48) of
each 64-partition strip, so output HBM traffic is exactly the
512x512 bf16 image.
"""

import numpy as np

B, H, W = 128, 512, 512
KS = 7
PAD = (KS - 1) // 2          # 3
HP = H + 2 * PAD             # 518
N_CORES = 8
PER_CORE = B // N_CORES      # 16
TS = 58                      # output rows per 64x64 tile (64 - 6)
TAILM = H - 8 * TS           # 48 tail output rows per image
TAILK = TAILM + KS - 1       # 54
NWARM = 9                    # dummy matmuls for PE clock warm-up


def _build_program():
    import concourse.bass as bass
    import concourse.tile as tile
    from concourse import bacc, mybir

    f16 = mybir.dt.float16
    bf16 = mybir.dt.bfloat16
    f32 = mybir.dt.float32

    nc = bacc.Bacc("TRN2", target_bir_lowering=False, debug=False,
                   num_devices=N_CORES)
    x_ext = nc.declare_dram_parameter("x", [PER_CORE, 128, 4 * HP], f16,
                                      isOutput=False)
    xt_ext = nc.declare_dram_parameter("xt", [PER_CORE, TAILK, HP], f16,
                                       isOutput=False)
    t_ext = nc.declare_dram_parameter("toep", [128, KS * TS], f16,
                                      isOutput=False)
    # og[img, g, p, 2s+r, w]: out row 232s + 116g + 58r + p  (p < 58)
    og_ext = nc.declare_dram_parameter("og", [PER_CORE, 2, TS, 4, W],
                                       bf16, isOutput=True)
    # ot[tg, g, p, r, w]: img 4tg + 2g + r, out row 464 + p  (p < 48)
    ot_ext = nc.declare_dram_parameter("ot", [PER_CORE // 4, 2, TAILM, 2, W],
                                       bf16, isOutput=True)

    with tile.TileContext(nc) as tc:
        with (
            tc.tile_pool(name="toep", bufs=1) as toep_pool,
            tc.tile_pool(name="xa", bufs=4) as xa_pool,
            tc.tile_pool(name="xb", bufs=8) as xb_pool,
            tc.tile_pool(name="xt", bufs=8) as xt_pool,
            tc.tile_pool(name="psum", bufs=8, space="PSUM") as psum_pool,
            tc.tile_pool(name="outs", bufs=3) as out_pool,
            tc.tile_pool(name="outt", bufs=2) as outt_pool,
            tc.tile_pool(name="scr", bufs=1) as scr_pool,
        ):
            # PE warm-up: keep the array busy through the HAM activity
            # window while the first input DMAs are in flight.
            scr = scr_pool.tile([128, W], f16, name="scr")
            nc.vector.memset(scr[:], 0.0)
            wps = psum_pool.tile([128, W], f32, name="wps", tag="acc")
            for i in range(NWARM):
                nc.tensor.matmul(
                    wps[0:TS, :], scr[0:64, 0:TS], scr[0:64, :],
                    start=True, stop=True, tile_position=(0, 0),
                )

            toep_sb = toep_pool.tile([128, KS * TS], f16)
            nc.sync.dma_start(out=toep_sb[:], in_=t_ext[:])

            def evac(ps, osb, blk, idx):
                if idx % 2 == 0:
                    nc.vector.tensor_copy(osb[:, W * blk:W * (blk + 1)],
                                          ps[:])
                else:
                    nc.scalar.copy(osb[:, W * blk:W * (blk + 1)], ps[:])

            stages = {}
            tails = {}
            for img in range(PER_CORE):
                # (partition 64r+p, slab q) = padded row 116q + 58r + p.
                # Slabs 0-1 and 2-3 are separate tiles so the s=0 matmuls
                # depend only on the first (smaller) load.
                st_a = xa_pool.tile([128, 2 * HP], f16, name="sta",
                                    tag="sta")
                nc.sync.dma_start(out=st_a[:], in_=x_ext[img, :, :2 * HP])
                st_b = xb_pool.tile([128, 2 * HP], f16, name="stb",
                                    tag="stb")
                nc.sync.dma_start(out=st_b[:], in_=x_ext[img, :, 2 * HP:])
                # tail rows 464..518 on partition strip img%2 only
                st_t = xt_pool.tile([128, HP], f16, name="stt", tag="stt")
                hh = 64 * (img % 2)
                nc.gpsimd.dma_start(out=st_t[hh:hh + TAILK, :],
                                    in_=xt_ext[img])
                stages[img] = st_b
                tails[img] = st_t

                osb = out_pool.tile([128, 4 * W], bf16, name="o",
                                    tag="osb")
                # s-groups sequential: only 2 PSUM banks live per group,
                # so allocation never stalls on evacuation of 4 banks.
                for s in range(2):
                    stage = st_a if s == 0 else st_b
                    ps = [psum_pool.tile([128, W], f32, name=f"ps{r}",
                                         tag="acc") for r in range(2)]
                    for v in range(KS):
                        for g in range(2):
                            for r in range(2):
                                nc.tensor.matmul(
                                    ps[r][64 * g:64 * g + TS, :],
                                    toep_sb[64 * r:64 * r + 64,
                                            TS * v:TS * (v + 1)],
                                    stage[64 * r:64 * r + 64,
                                          g * HP + v:g * HP + v + W],
                                    start=(v == 0),
                                    stop=(v == KS - 1),
                                    tile_position=(64 * r, 64 * g),
                                )
                    for r in range(2):
                        evac(ps[r], osb, 2 * s + r, img * 4 + 2 * s + r)

                    if s == 0 and img % 4 == 3:
                        # tail group between the two s-groups so its
                        # evacuations interleave with s=1 compute
                        tg = img // 4
                        pst = [psum_pool.tile([128, W], f32,
                                              name=f"pt{r}", tag="acc")
                               for r in range(2)]
                        for v in range(KS):
                            for j in range(4):
                                r, g = j % 2, j // 2
                                nc.tensor.matmul(
                                    pst[r][64 * g:64 * g + TAILM, :],
                                    toep_sb[64 * r:64 * r + TAILK,
                                            TS * v:TS * v + TAILM],
                                    tails[4 * tg + j][
                                        64 * r:64 * r + TAILK,
                                        v:v + W],
                                    start=(v == 0),
                                    stop=(v == KS - 1),
                                    tile_position=(64 * r, 64 * g),
                                )
                        otb = outt_pool.tile([128, 2 * W], bf16,
                                             name="p", tag="otb")
                        for r in range(2):
                            evac(pst[r], otb, r, img * 4 + r)
                        for g in range(2):
                            ring = nc.gpsimd if g == 0 else nc.sync
                            ring.dma_start(
                                out=ot_ext[tg, g],
                                in_=otb[64 * g:64 * g + TAILM, :])
                        stages = {}
                        tails = {}

                # keep DMA issue off ScalarE/VectorE: a store issue that
                # blocks on ring-completion sems would delay the PSUM
                # evacuation copies queued behind it and stall the PE
                for g in range(2):
                    ring = nc.gpsimd if (img + g) % 2 == 0 else nc.sync
                    ring.dma_start(out=og_ext[img, g],
                                   in_=osb[64 * g:64 * g + TS, :])
    nc.finalize()
    return nc


def _host_prep(x, w):
    x = np.asarray(x, dtype=np.float32)
    w = np.asarray(w, dtype=np.float32)
    xpad = np.zeros((B, HP, HP), dtype=np.float16)
    xpad[:, PAD:PAD + H, PAD:PAD + W] = x
    # main slabs: (p, q<4) -> padded row 116q + 58*(p//64) + p%64
    p = np.arange(128)
    q = np.arange(4)
    ridx = 116 * q[None, :] + 58 * (p[:, None] // 64) + (p[:, None] % 64)
    xmain = np.ascontiguousarray(
        xpad[:, ridx, :].reshape(B, 128, 4 * HP))
    # tail rows 464..517 (54 = TAILK), compact
    xtail = np.ascontiguousarray(xpad[:, 464:464 + TAILK, :])
    # Toeplitz band [64, 58] per tap, replicated on both partition strips
    toep = np.zeros((128, KS * TS), dtype=np.float16)
    w16 = w.astype(np.float16)
    idx = np.arange(TS)
    for st in range(2):
        for v in range(KS):
            for d in range(KS):
                toep[64 * st + idx + d, TS * v + idx] = w16[d, v]
    return xmain, xtail, toep


def _execute(x, w, **run_kwargs):
    from concourse.bass_utils import run_bass_kernel_spmd

    xmain, xtail, toep = _host_prep(x, w)
    nc = _build_program()
    in_maps = [
        {"x": xmain[c * PER_CORE:(c + 1) * PER_CORE],
         "xt": xtail[c * PER_CORE:(c + 1) * PER_CORE],
         "toep": toep}
        for c in range(N_CORES)
    ]
    last_err = None
    for _attempt in range(3):
        try:
            res = run_bass_kernel_spmd(nc, in_maps,
                                       core_ids=list(range(N_CORES)),
                                       **run_kwargs)
            break
        except Exception as e:  # transient NRT execute flakes -> retry
            last_err = e
    else:
        raise last_err

    # og[img, g, p, 2s+r, w] -> out row 232s + 116g + 58r + p
    g_i, p_i, b_i = np.meshgrid(np.arange(2), np.arange(TS), np.arange(4),
                                indexing="ij")
    rm_og = (232 * (b_i // 2) + 116 * g_i + 58 * (b_i % 2) + p_i).ravel()
    out = np.empty((B, H, W), dtype=np.float32)
    for c in range(N_CORES):
        sl = slice(c * PER_CORE, (c + 1) * PER_CORE)
        og = np.asarray(res.results[c]["og"], dtype=np.float32)
        ot = np.asarray(res.results[c]["ot"], dtype=np.float32)
        out[sl][:, rm_og, :] = og.reshape(PER_CORE, 2 * TS * 4, W)
        # ot[tg, g, p, r, w] -> img 4tg + 2g + r, row 464 + p
        out[sl, 8 * TS:, :] = ot.transpose(0, 1, 3, 2, 4).reshape(
            PER_CORE, TAILM, W)
    return out, res


def kernel(x, w):
    out, _ = _execute(x, w)
    return out


# revision 12
# speedup vs baseline: 1.3251x; 1.2318x over previous
"""7x7 'same' 2D convolution over [128, 512, 512] f32, data-parallel on 8 NeuronCores.

Banded-Toeplitz formulation on the TensorEngine with 64x64 array
packing: the PE array runs as 4 independent 64x64 tiles
(tile_position=(64r, 64g)), each computing a 58-row output block
    out[i0+m, j] = sum_v sum_{r'} T_v[r', m] * xpad[i0+r', j+v]
with T_v[r', m] = w[r'-m, v] (band, r'<64, m<58). The 7 column taps (v)
accumulate into PSUM; tile (s, r2, g2) covers out rows
232s + 116g2 + 58r2 + [0,58) and writes PSUM bank (s, r2), partitions
[64g2, 64g2+58). One 4-tile tap sweep streams in one N=512 matmul
time; 8 tiles cover rows 0..463 of an image. Rows 464..511 of four
consecutive images batch into one 4-tile "tail" group whose moving
data sits on partition strip j%2 only (no duplicated tail rows).

A burst of dummy matmuls on scratch data runs during the initial DMA
lead-in so the PE HAM clock-gate reaches 8/8 (2.4 GHz) before the
first real matmul; without it the first ~4us of real matmuls run at
1.2 GHz.

Inputs are cast to fp16 and pre-staged host-side into the SBUF slab
layout (partition 64r+p, slab q<4, col) = padded row 116q + 58r + p;
the 54 tail rows load separately into partitions [64*(img%2), +54).
Accumulation is fp32; PSUM banks are evacuated full-width (VectorE/
ScalarE alternating) into [128, 2048] bf16 staging tiles and stored
with plain partition-range DMAs that carry only the valid 58 (48) of
each 64-partition strip, so output HBM traffic is exactly the
512x512 bf16 image.
"""

import numpy as np

B, H, W = 128, 512, 512
KS = 7
PAD = (KS - 1) // 2          # 3
HP = H + 2 * PAD             # 518
N_CORES = 8
PER_CORE = B // N_CORES      # 16
TS = 58                      # output rows per 64x64 tile (64 - 6)
TAILM = H - 8 * TS           # 48 tail output rows per image
TAILK = TAILM + KS - 1       # 54
NWARM = 9                    # dummy matmuls for PE clock warm-up


def _build_program():
    import concourse.bass as bass
    import concourse.tile as tile
    from concourse import bacc, mybir

    f16 = mybir.dt.float16
    bf16 = mybir.dt.bfloat16
    f32 = mybir.dt.float32

    nc = bacc.Bacc("TRN2", target_bir_lowering=False, debug=False,
                   num_devices=N_CORES)
    x_ext = nc.declare_dram_parameter("x", [PER_CORE, 128, 4 * HP], f16,
                                      isOutput=False)
    xt_ext = nc.declare_dram_parameter("xt", [PER_CORE, TAILK, HP], f16,
                                       isOutput=False)
    t_ext = nc.declare_dram_parameter("toep", [128, KS * TS], f16,
                                      isOutput=False)
    # og[img, g, p, 2s+r, w]: out row 232s + 116g + 58r + p  (p < 58)
    og_ext = nc.declare_dram_parameter("og", [PER_CORE, 2, TS, 4, W],
                                       bf16, isOutput=True)
    # ot[tg, g, p, r, w]: img 4tg + 2g + r, out row 464 + p  (p < 48)
    ot_ext = nc.declare_dram_parameter("ot", [PER_CORE // 4, 2, TAILM, 2, W],
                                       bf16, isOutput=True)

    with tile.TileContext(nc) as tc:
        with (
            tc.tile_pool(name="toep", bufs=1) as toep_pool,
            tc.tile_pool(name="xa", bufs=4) as xa_pool,
            tc.tile_pool(name="xb", bufs=8) as xb_pool,
            tc.tile_pool(name="xt", bufs=8) as xt_pool,
            tc.tile_pool(name="psum", bufs=8, space="PSUM") as psum_pool,
            tc.tile_pool(name="outs", bufs=3) as out_pool,
            tc.tile_pool(name="outt", bufs=2) as outt_pool,
            tc.tile_pool(name="scr", bufs=1) as scr_pool,
        ):
            # PE warm-up: keep the array busy through the HAM activity
            # window while the first input DMAs are in flight.
            scr = scr_pool.tile([128, W], f16, name="scr")
            nc.vector.memset(scr[:], 0.0)
            wps = psum_pool.tile([128, W], f32, name="wps", tag="acc")
            for i in range(NWARM):
                nc.tensor.matmul(
                    wps[0:TS, :], scr[0:64, 0:TS], scr[0:64, :],
                    start=True, stop=True, tile_position=(0, 0),
                )

            toep_sb = toep_pool.tile([128, KS * TS], f16)
            nc.sync.dma_start(out=toep_sb[:], in_=t_ext[:])

            def evac(ps, osb, blk, idx):
                if idx % 2 == 0:
                    nc.vector.tensor_copy(osb[:, W * blk:W * (blk + 1)],
                                          ps[:])
                else:
                    nc.scalar.copy(osb[:, W * blk:W * (blk + 1)], ps[:])

            stages = {}
            tails = {}
            for img in range(PER_CORE):
                # (partition 64r+p, slab q) = padded row 116q + 58r + p.
                # Slabs 0-1 and 2-3 are separate tiles so the s=0 matmuls
                # depend only on the first (smaller) load.
                st_a = xa_pool.tile([128, 2 * HP], f16, name="sta",
                                    tag="sta")
                nc.sync.dma_start(out=st_a[:], in_=x_ext[img, :, :2 * HP])
                st_b = xb_pool.tile([128, 2 * HP], f16, name="stb",
                                    tag="stb")
                nc.sync.dma_start(out=st_b[:], in_=x_ext[img, :, 2 * HP:])
                # tail rows 464..518 on partition strip img%2 only
                st_t = xt_pool.tile([128, HP], f16, name="stt", tag="stt")
                hh = 64 * (img % 2)
                nc.sync.dma_start(out=st_t[hh:hh + TAILK, :],
                                  in_=xt_ext[img])
                stages[img] = st_b
                tails[img] = st_t

                osb = out_pool.tile([128, 4 * W], bf16, name="o",
                                    tag="osb")
                # s-groups sequential: only 2 PSUM banks live per group,
                # so allocation never stalls on evacuation of 4 banks.
                for s in range(2):
                    stage = st_a if s == 0 else st_b
                    ps = [psum_pool.tile([128, W], f32, name=f"ps{r}",
                                         tag="acc") for r in range(2)]
                    for v in range(KS):
                        for g in range(2):
                            for r in range(2):
                                nc.tensor.matmul(
                                    ps[r][64 * g:64 * g + TS, :],
                                    toep_sb[64 * r:64 * r + 64,
                                            TS * v:TS * (v + 1)],
                                    stage[64 * r:64 * r + 64,
                                          g * HP + v:g * HP + v + W],
                                    start=(v == 0),
                                    stop=(v == KS - 1),
                                    tile_position=(64 * r, 64 * g),
                                )
                    for r in range(2):
                        evac(ps[r], osb, 2 * s + r, img * 4 + 2 * s + r)

                    if s == 0 and img % 4 == 3:
                        # tail group between the two s-groups so its
                        # evacuations interleave with s=1 compute
                        tg = img // 4
                        pst = [psum_pool.tile([128, W], f32,
                                              name=f"pt{r}", tag="acc")
                               for r in range(2)]
                        for v in range(KS):
                            for j in range(4):
                                r, g = j % 2, j // 2
                                nc.tensor.matmul(
                                    pst[r][64 * g:64 * g + TAILM, :],
                                    toep_sb[64 * r:64 * r + TAILK,
                                            TS * v:TS * v + TAILM],
                                    tails[4 * tg + j][
                                        64 * r:64 * r + TAILK,
                                        v:v + W],
                                    start=(v == 0),
                                    stop=(v == KS - 1),
                                    tile_position=(64 * r, 64 * g),
                                )
                        otb = outt_pool.tile([128, 2 * W], bf16,
                                             name="p", tag="otb")
                        for r in range(2):
                            evac(pst[r], otb, r, img * 4 + r)
                        for g in range(2):
                            nc.gpsimd.dma_start(
                                out=ot_ext[tg, g],
                                in_=otb[64 * g:64 * g + TAILM, :])
                        stages = {}
                        tails = {}

                # Ring discipline: sync carries only loads, gpsimd only
                # stores, ScalarE/VectorE only PSUM evacuation copies. A
                # store issue can block on ring-completion semaphores near
                # HBM saturation; anything queued behind it on the same
                # engine (loads, evac copies) would stall the PE.
                for g in range(2):
                    nc.gpsimd.dma_start(out=og_ext[img, g],
                                        in_=osb[64 * g:64 * g + TS, :])
    nc.finalize()
    return nc


def _host_prep(x, w):
    x = np.asarray(x, dtype=np.float32)
    w = np.asarray(w, dtype=np.float32)
    xpad = np.zeros((B, HP, HP), dtype=np.float16)
    xpad[:, PAD:PAD + H, PAD:PAD + W] = x
    # main slabs: (p, q<4) -> padded row 116q + 58*(p//64) + p%64
    p = np.arange(128)
    q = np.arange(4)
    ridx = 116 * q[None, :] + 58 * (p[:, None] // 64) + (p[:, None] % 64)
    xmain = np.ascontiguousarray(
        xpad[:, ridx, :].reshape(B, 128, 4 * HP))
    # tail rows 464..517 (54 = TAILK), compact
    xtail = np.ascontiguousarray(xpad[:, 464:464 + TAILK, :])
    # Toeplitz band [64, 58] per tap, replicated on both partition strips
    toep = np.zeros((128, KS * TS), dtype=np.float16)
    w16 = w.astype(np.float16)
    idx = np.arange(TS)
    for st in range(2):
        for v in range(KS):
            for d in range(KS):
                toep[64 * st + idx + d, TS * v + idx] = w16[d, v]
    return xmain, xtail, toep


def _execute(x, w, **run_kwargs):
    from concourse.bass_utils import run_bass_kernel_spmd

    xmain, xtail, toep = _host_prep(x, w)
    nc = _build_program()
    in_maps = [
        {"x": xmain[c * PER_CORE:(c + 1) * PER_CORE],
         "xt": xtail[c * PER_CORE:(c + 1) * PER_CORE],
         "toep": toep}
        for c in range(N_CORES)
    ]
    last_err = None
    for _attempt in range(3):
        try:
            res = run_bass_kernel_spmd(nc, in_maps,
                                       core_ids=list(range(N_CORES)),
                                       **run_kwargs)
            break
        except Exception as e:  # transient NRT execute flakes -> retry
            last_err = e
    else:
        raise last_err

    # og[img, g, p, 2s+r, w] -> out row 232s + 116g + 58r + p
    g_i, p_i, b_i = np.meshgrid(np.arange(2), np.arange(TS), np.arange(4),
                                indexing="ij")
    rm_og = (232 * (b_i // 2) + 116 * g_i + 58 * (b_i % 2) + p_i).ravel()
    out = np.empty((B, H, W), dtype=np.float32)
    for c in range(N_CORES):
        sl = slice(c * PER_CORE, (c + 1) * PER_CORE)
        og = np.asarray(res.results[c]["og"], dtype=np.float32)
        ot = np.asarray(res.results[c]["ot"], dtype=np.float32)
        out[sl][:, rm_og, :] = og.reshape(PER_CORE, 2 * TS * 4, W)
        # ot[tg, g, p, r, w] -> img 4tg + 2g + r, row 464 + p
        out[sl, 8 * TS:, :] = ot.transpose(0, 1, 3, 2, 4).reshape(
            PER_CORE, TAILM, W)
    return out, res


def kernel(x, w):
    out, _ = _execute(x, w)
    return out
